# revision 61
# baseline (speedup 1.0000x reference)
"""AnchorLoss distributed Trainium2 kernel (8 NeuronCores).

reference math (anchors: [8192, 8, 512] f32):
    x = anchors.reshape(8192, 4096)
    loss = -(2*N*sum(x*x) - 2*sum(colsum(x)^2)) / sqrt(512)

Strategy: shard COLUMNS across the 8 cores (512 columns each). Each core
streams its [8192, 512] column slice (16 MiB) in 16 tiles of
[128, 4x512] (4 row-blocks per tile) over TWO parallel DMA rings:
5 tiles stay f32 on the SP HWDGE ring; 11 tiles are converted
f32->bf16 inside the gpsimd SWDGE DMA (HBM still reads every f32 byte
once - bf16 is an internal compute-precision choice). Per tile:
  - partial sum of squares, load-balanced across ScalarE
    (Square + accum_out) and VectorE (2x bf16 tensor_mul, then a x1.0
    tensor_scalar whose accum_out reduces at 4x)
  - the COMPLETE column sums of its 512 columns via PE matmuls
    (lhsT = x block [128,128], rhs = ones [128,1], PSUM-accumulated
    over the 4 row-blocks, then SBUF-accumulated over tiles)
so the only cross-core data is one scalar per core:
    c_k = (2/f)*||colsum_k||^2 - (2*N/f)*sumsq_k
Each core replicates c_k 8x and a ReduceScatter-add leaves
loss = sum_k c_k (= -total/f) in every core's [1] bounce buffer;
a DRAM->DRAM copy lands it in "out". Host takes core 0's scalar.
"""

import numpy as np

from concourse import bacc, tile, mybir
from concourse.bass_utils import run_bass_kernel_spmd

N_CORES = 8
N_CLASSES = 8192
D = 4096                        # 8 * 512 flattened embedding dim
COLS = D // N_CORES             # 512 columns per core
P = 128                         # partitions
RB = 4                          # row-blocks per tile
TILE_ROWS = P * RB              # 512 rows per tile
N_TILES = N_CLASSES // TILE_ROWS  # 16
CHUNK = 128                     # columns per colsum matmul
N_CHUNKS = COLS // CHUNK        # 4
FACTOR = float(np.sqrt(np.float32(512.0)))


def _build():
    nc = bacc.Bacc(None, num_devices=N_CORES)
    x_ext = nc.declare_dram_parameter(
        "anchors", [N_CLASSES, COLS], mybir.dt.float32, isOutput=False
    )
    out_ext = nc.declare_dram_parameter(
        "out", [1, 1], mybir.dt.float32, isOutput=True
    )

    with tile.TileContext(nc) as tc:
        with (
            tc.tile_pool(name="io", bufs=6) as io,
            tc.tile_pool(name="small", bufs=1) as sp,
            tc.tile_pool(name="psum", bufs=1, space="PSUM") as ps,
            tc.tile_pool(name="dram", bufs=1, space="DRAM") as dr,
        ):
            ones = sp.tile([P, 1], mybir.dt.float32)
            nc.gpsimd.memset(ones[:], 1.0)
            ones_bf = sp.tile([P, 1], mybir.dt.bfloat16)
            nc.gpsimd.memset(ones_bf[:], 1.0)
            # one accum column per (tile, sub-square): the last two tiles
            # split their square into RB chunks to shorten the critical tail
            rowsumsq = sp.tile([P, N_TILES + 2 * (RB - 1)], mybir.dt.float32)
            scr_s = sp.tile([P, RB, COLS], mybir.dt.float32)
            scr_sb = sp.tile([P, RB, COLS], mybir.dt.bfloat16)
            scr_vu = sp.tile([P, COLS], mybir.dt.bfloat16)
            scr_vb = sp.tile([P, RB, COLS], mybir.dt.bfloat16)
            cs_acc = sp.tile([P, N_CHUNKS], mybir.dt.float32)
            nc.vector.memset(cs_acc[:], 0.0)

            for t in range(N_TILES):
                # alternate tiles between the SP HWDGE ring (f32) and the
                # gpsimd SWDGE ring (converted f32->bf16 in the DMA) so the
                # two DMA FIFOs stream in parallel and bf16 tiles square
                # at 2x on ACT/DVE. HBM still reads every f32 byte once.
                bf = t not in (0, 3, 6, 9, 12)
                dt_t = mybir.dt.bfloat16 if bf else mybir.dt.float32
                dma_eng = nc.gpsimd if bf else nc.sync
                one_t = ones_bf if bf else ones
                xt = io.tile([P, RB, COLS], dt_t,
                             tag="xtb" if bf else "xt", name=f"xt{t}")
                src = x_ext[t * TILE_ROWS:(t + 1) * TILE_ROWS, :]
                src = src.rearrange("(rb p) c -> p rb c", rb=RB, p=P)
                # the last two tiles are DMA'd and squared per row-block so
                # only a short square trails the final DMA
                if t < N_TILES - 2:
                    dma_eng.dma_start(xt[:], src)
                    if t in (1, 2, 4, 5, 7, 8, 10, 13):
                        # bf16 full squares on DVE: 2x mult, then a x1.0
                        # tensor_scalar whose accum_out sums at 4x
                        nc.vector.tensor_mul(scr_vb[:], xt[:], xt[:])
                        nc.vector.tensor_scalar(
                            scr_vb[:], scr_vb[:], 1.0, None,
                            mybir.AluOpType.mult, mybir.AluOpType.add,
                            accum_out=rowsumsq[:, t:t + 1],
                        )
                    else:
                        # the rest on ScalarE
                        scr = scr_sb if bf else scr_s
                        nc.scalar.activation(
                            scr[:], xt[:],
                            mybir.ActivationFunctionType.Square,
                            accum_out=rowsumsq[:, t:t + 1],
                        )
                else:
                    base = t + (t - (N_TILES - 2)) * (RB - 1)
                    dma_eng.dma_start(xt[:], src)
                    for j in range(RB):
                        col = rowsumsq[:, base + j:base + j + 1]
                        if (t, j) in ((N_TILES - 2, 0), (N_TILES - 2, 1),
                                      (N_TILES - 2, 3), (N_TILES - 1, 1),
                                      (N_TILES - 1, 3)):
                            # some unit pairs on DVE
                            nc.vector.tensor_mul(scr_vu[:], xt[:, j, :],
                                                 xt[:, j, :])
                            nc.vector.tensor_scalar(
                                scr_vu[:], scr_vu[:], 1.0, None,
                                mybir.AluOpType.mult, mybir.AluOpType.add,
                                accum_out=col,
                            )
                        else:
                            # f32 units j0/j2 and all bf16 units on ScalarE
                            scr = scr_sb if bf else scr_s
                            nc.scalar.activation(
                                scr[:, j, :], xt[:, j, :],
                                mybir.ActivationFunctionType.Square,
                                accum_out=col,
                            )
                # column sums of this tile's 512 rows:
                # cs_ps[m, c] = sum_{rb,p} xt[p, rb, c*128+m]
                cs_ps = ps.tile(
                    [P, N_CHUNKS], mybir.dt.float32, tag="cs_ps",
                    name=f"cs{t}", bufs=2,
                )
                for c in range(N_CHUNKS):
                    for j in range(RB):
                        nc.tensor.matmul(
                            cs_ps[:, c:c + 1],
                            lhsT=xt[:, j, c * CHUNK:(c + 1) * CHUNK],
                            rhs=one_t[:],
                            start=(j == 0), stop=(j == RB - 1),
                        )
                nc.vector.tensor_add(cs_acc[:], cs_acc[:], cs_ps[:])

            # local scalars: F[:,0] = per-partition sumsq, F[:,1] = colsum^2
            F = sp.tile([P, 2], mybir.dt.float32)
            nc.vector.tensor_reduce(
                out=F[:, 0:1], in_=rowsumsq[:],
                axis=mybir.AxisListType.X, op=mybir.AluOpType.add,
            )
            # colsum^2 on DVE (keeps it off ScalarE's tail queue)
            scr2 = sp.tile([P, N_CHUNKS], mybir.dt.float32)
            nc.vector.tensor_mul(scr2[:], cs_acc[:], cs_acc[:])
            nc.vector.tensor_reduce(
                out=F[:, 1:2], in_=scr2[:],
                axis=mybir.AxisListType.X, op=mybir.AluOpType.add,
            )
            res_ps = ps.tile([1, 2], mybir.dt.float32)
            nc.tensor.matmul(res_ps[:], lhsT=ones[:], rhs=F[:],
                             start=True, stop=True)
            # c_k = (2/f)*colsumsq_k - (2*N/f)*sumsq_k
            a_sb = sp.tile([1, 1], mybir.dt.float32)
            nc.vector.tensor_scalar_mul(
                a_sb[:], res_ps[0:1, 0:1], float(2.0 * N_CLASSES / FACTOR)
            )
            ck_sb = sp.tile([1, 1], mybir.dt.float32)
            nc.vector.scalar_tensor_tensor(
                out=ck_sb[:], in0=res_ps[0:1, 1:2],
                scalar=float(2.0 / FACTOR), in1=a_sb[:],
                op0=mybir.AluOpType.mult, op1=mybir.AluOpType.subtract,
            )

            # sum the 8 per-core scalars: replicate ck 8x, ReduceScatter-add
            # -> each core's [1] output IS the loss; copy DRAM->DRAM to out
            ck8 = sp.tile([1, N_CORES], mybir.dt.float32)
            nc.vector.tensor_copy(ck8[:], ck_sb[:].broadcast_to([1, N_CORES]))
            cc_in = dr.tile([N_CORES], mybir.dt.float32)
            cc_out = dr.tile([1], mybir.dt.float32)
            nc.sync.dma_start(cc_in[:], ck8[:])
            nc.gpsimd.collective_compute(
                "ReduceScatter",
                mybir.AluOpType.add,
                replica_groups=[list(range(N_CORES))],
                ins=[cc_in[:]],
                outs=[cc_out[:]],
            )
            nc.sync.dma_start(out_ext[:], cc_out[:])
    nc.finalize()
    return nc


_NC_CACHE = None


def _get_nc():
    global _NC_CACHE
    if _NC_CACHE is None:
        _NC_CACHE = _build()
    return _NC_CACHE


def _run(anchors: np.ndarray, trace: bool = False):
    """Returns (loss_scalar, BassKernelResults)."""
    x = np.asarray(anchors, dtype=np.float32).reshape(N_CLASSES, D)
    in_maps = [
        {"anchors": np.ascontiguousarray(x[:, i * COLS:(i + 1) * COLS])}
        for i in range(N_CORES)
    ]
    nc = _get_nc()
    res = run_bass_kernel_spmd(nc, in_maps, core_ids=list(range(N_CORES)), trace=trace)
    loss = np.float32(np.asarray(res.results[0]["out"]).reshape(())[()])
    return loss, res


def kernel(anchors: np.ndarray) -> np.ndarray:
    loss, _ = _run(anchors)
    return np.asarray(loss, dtype=np.float32).reshape(())
